# revision 5
# baseline (speedup 1.0000x reference)
"""Weighted Chamfer loss on Trainium2 (8 NeuronCores, batch-parallel).

Problem (per batch element b of 8):
    dist[i, j] = || set1[b, i] - set2[b, j] ||_2            (4096 x 4096, C=128)
    total = (sum_i w1[b,i] * min_j dist + sum_j w2[b,j] * min_i dist) / 2

Sharding: one batch element per NeuronCore (pure data parallel, no
collectives); the 8 per-core partial sums are added on the host.

Kernel strategy (vs the fp16 baseline):
  PE    : fp8e4 DoubleRow matmuls, K = 2 k-tiles x 66 rows = 128 channels
          + 4 spare rank-1 rows that bake -x2/2 (+ fp8 residual row) and
          -y2/2 (+ residual) into the same pass -> PSUM = -d^2/2 complete,
          at 0.5 cyc/row. One [128 x 4096] PSUM unit per x-block (all 8
          banks, 2-deep ping-pong), 8 matmuls of 512 cols each.
  ACT   : evacuates the unit with Identity(scale=-2) -> d2 fp16.
  DVE   : one fp16 tensor_reduce(min) over the unit -> row-min slot, and
          the column-min accumulator update (fp16 tensor_tensor min).
  Tail  : PE transposes of colacc + strided min reduce -> per-column mins;
          relu+sqrt; weighted sums; host adds the 8 per-core scalars.

The norms are computed from the *quantized* fp8 values (squares of fp8
are exact in fp16), so PSUM holds exactly -||x_q - y_q||^2/2 up to the
fp8 residual-row rounding (|err| <= ~0.25 on d2 values of ~170).
"""

import sys
from contextlib import ExitStack, nullcontext

import numpy as np

for _p in ("/opt/trn_rl_repo",):
    if _p not in sys.path:
        sys.path.insert(0, _p)

import concourse.bass as bass
import concourse.tile as tile
from concourse import bacc, masks, mybir
from concourse.bass_utils import run_bass_kernel_spmd

AF = mybir.ActivationFunctionType
ALU = mybir.AluOpType
DT = mybir.dt
PM = mybir.MatmulPerfMode

N_CORES = 8
N = 4096          # points per set per batch element
C = 128           # channels (contraction dim)
KP = C // 2 + 2   # 66: contraction rows per DoubleRow k-tile (64 ch + 2 bake)
NB = N // 128     # 32 row blocks of x
MMN = 512         # moving free dim per matmul (one fp32 PSUM bank)
NT = N // 128     # 32 transpose tiles

_CACHE = {}
LAST_RESULTS = None  # BassKernelResults of the most recent run (for profiling)


def _build_program(repeat=1, parts="pe,act,dve"):
    # tuning knob: "eNNNN" = ACT evac columns (rest go to DVE tensor_scalar)
    EA = N
    for p in parts.split(","):
        if p.startswith("e") and p[1:].isdigit():
            EA = int(p[1:])
    en_act = "act" in parts
    en_dve = "dve" in parts

    nc = bacc.Bacc(
        "TRN2", debug=False, target_bir_lowering=False, num_devices=N_CORES
    )
    xt_d = nc.dram_tensor("xt", [C, N], DT.float32, kind="ExternalInput").ap()
    yt_d = nc.dram_tensor("yt", [C, N], DT.float32, kind="ExternalInput").ap()
    w1t_d = nc.dram_tensor("w1t", [128, NB], DT.float32, kind="ExternalInput").ap()
    w2t_d = nc.dram_tensor("w2t", [128, NB], DT.float32, kind="ExternalInput").ap()
    out_d = nc.dram_tensor("out", [128, 2], DT.float32, kind="ExternalOutput").ap()

    with tile.TileContext(nc) as tc, ExitStack() as ctx:
        persist = ctx.enter_context(tc.tile_pool(name="persist", bufs=1))
        prep = ctx.enter_context(tc.tile_pool(name="prep", bufs=1))
        rows = ctx.enter_context(tc.tile_pool(name="rows", bufs=1))
        d2p = ctx.enter_context(tc.tile_pool(name="d2", bufs=4))
        psum = ctx.enter_context(tc.tile_pool(name="psum", bufs=2, space="PSUM"))

        # ---------------- persistent tiles ----------------
        # DoubleRow operands [KP, 2, N] stored as [KP, 2*N]:
        #   x tile0 rows: ch 0..63, then -x2/2 (fp8), res_x (fp8)
        #   x tile1 rows: ch 64..127, then 1, 1
        #   y tile0 rows: ch 0..63, then 1, 1
        #   y tile1 rows: ch 64..127, then -y2/2, res_y
        xq8 = persist.tile([KP, 2 * N], DT.float8e4)
        yq8 = persist.tile([KP, 2 * N], DT.float8e4)

        identity = persist.tile([128, 128], DT.float16)
        masks.make_identity(nc, identity[:])

        ones = persist.tile([C, 1], DT.float16)
        nc.gpsimd.memset(ones[:], 1.0)

        colacc = persist.tile([128, N], DT.float16)
        nc.gpsimd.memset(colacc[:], 60000.0)

        rm = persist.tile([128, NB], DT.float32)
        colminT = persist.tile([128, NB], DT.float32)

        w1t = persist.tile([128, NB], DT.float32)
        w2t = persist.tile([128, NB], DT.float32)
        nc.sync.dma_start(w1t[:], w1t_d[:])
        nc.sync.dma_start(w2t[:], w2t_d[:])

        # ---------------- prep: quantize + norms + assemble ----------------
        # spare rows default to 1.0; the norm rows overwrite their halves
        nc.vector.memset(xq8[C // 2 : KP, :], 1.0)
        nc.vector.memset(yq8[C // 2 : KP, :], 1.0)

        for src_d, q8big, half in ((xt_d, xq8, 0), (yt_d, yq8, 1)):
            stage = prep.tile([C, N], DT.float32, tag="stage")
            nc.sync.dma_start(stage[:], src_d[:])
            qfull = prep.tile([C, N], DT.float8e4, tag="qfull")
            nc.vector.tensor_copy(qfull[:], stage[:])
            h16 = prep.tile([C, N], DT.float16, tag="h16")
            nc.vector.tensor_copy(h16[:], qfull[:])
            sq = prep.tile([C, N], DT.float16, tag="sq")
            nc.scalar.activation(sq[:], h16[:], AF.Square)

            # -sum(sq)/2 over channels via ones-matmul into PSUM row 0
            mrow = rows.tile([1, N], DT.float32, tag="mrow")
            for hh in range(2):
                ps = psum.tile(
                    [128, N // 2], DT.float32, tag="unit", name=f"nps{half}{hh}"
                )
                for k in range(N // 2 // MMN):
                    c0 = k * MMN
                    nc.tensor.matmul(
                        ps[0:1, c0 : c0 + MMN],
                        ones[:],
                        sq[:, hh * (N // 2) + c0 : hh * (N // 2) + c0 + MMN],
                        start=True,
                        stop=True,
                    )
                nc.scalar.activation(
                    mrow[0:1, hh * (N // 2) : (hh + 1) * (N // 2)],
                    ps[0:1, :],
                    AF.Identity,
                    scale=-0.5,
                )

            m8 = rows.tile([1, N], DT.float8e4, tag="m8")
            nc.vector.tensor_copy(m8[:], mrow[:])
            mup = rows.tile([1, N], DT.float32, tag="mup")
            nc.vector.tensor_copy(mup[:], m8[:])
            resf = rows.tile([1, N], DT.float32, tag="resf")
            nc.vector.tensor_sub(resf[:], mrow[:], mup[:])
            r8 = rows.tile([1, N], DT.float8e4, tag="r8")
            nc.vector.tensor_copy(r8[:], resf[:])

            # assemble the [KP, 2, N] operand (SBUF->SBUF DMAs)
            nc.sync.dma_start(q8big[0 : C // 2, 0:N], qfull[0 : C // 2, :])
            nc.sync.dma_start(q8big[0 : C // 2, N : 2 * N], qfull[C // 2 : C, :])
            off = 0 if half == 0 else N
            nc.sync.dma_start(q8big[C // 2 : C // 2 + 1, off : off + N], m8[:])
            nc.sync.dma_start(q8big[C // 2 + 1 : KP, off : off + N], r8[:])

        xv = xq8[:].rearrange("p (two n) -> p two n", two=2)
        yv = yq8[:].rearrange("p (two n) -> p two n", two=2)

        if not en_dve:
            nc.gpsimd.memset(rm[:], 1.0)

        with tc.For_i(0, repeat, 1) if repeat > 1 else nullcontext():
            # ------- main loop: per x-block, two [128, 2048] PSUM units ------
            # evacuated into one [128, 4096] fp16 tile -> a single full-width
            # row-min reduce and a single full-width colacc min per block.
            for b in range(NB):
                d2 = d2p.tile([128, N], DT.float16, tag="d2")
                for h in range(2):
                    hc = h * (N // 2)
                    ps = psum.tile([128, N // 2], DT.float32, tag="unit")
                    for k in range(N // 2 // MMN):
                        c0 = k * MMN
                        nc.tensor.matmul(
                            ps[:, c0 : c0 + MMN],
                            xv[:, :, b * 128 : (b + 1) * 128],
                            yv[:, :, hc + c0 : hc + c0 + MMN],
                            start=True,
                            stop=True,
                            perf_mode=PM.DoubleRow,
                        )
                    if en_act:
                        ea = max(0, min(EA - hc, N // 2))
                        if ea > 0:
                            nc.scalar.activation(
                                d2[:, hc : hc + ea],
                                ps[:, 0:ea],
                                AF.Identity,
                                scale=-2.0,
                            )
                        if ea < N // 2:
                            nc.vector.tensor_scalar_mul(
                                d2[:, hc + ea : hc + N // 2],
                                ps[:, ea : N // 2],
                                -2.0,
                            )
                if en_dve:
                    nc.vector.tensor_reduce(
                        rm[:, b : b + 1],
                        d2[:],
                        axis=mybir.AxisListType.X,
                        op=ALU.min,
                    )
                    nc.vector.tensor_tensor(
                        colacc[:], d2[:], colacc[:], ALU.min
                    )

            # column mins: PE-transpose 128-blocks of colacc into PSUM
            # units, then one strided min-reduce per half.
            for h in range(2):
                hc = h * (N // 2)
                pst = psum.tile([128, N // 2], DT.float16, tag="unit", name="pst")
                for t in range(NT // 2):
                    nc.tensor.transpose(
                        pst[:, t * 128 : (t + 1) * 128],
                        colacc[:, hc + t * 128 : hc + (t + 1) * 128],
                        identity[:],
                    )
                nc.vector.tensor_reduce(
                    colminT[:, h * (NB // 2) : (h + 1) * (NB // 2)],
                    pst[:].rearrange("p (t c) -> p t c", c=128),
                    axis=mybir.AxisListType.X,
                    op=ALU.min,
                )

            # ---------------- tail ----------------
            rowr = persist.tile([128, NB], DT.float32)
            rowd = persist.tile([128, NB], DT.float32)
            nc.scalar.activation(rowr[:], rm[:], AF.Relu)
            nc.scalar.activation(rowd[:], rowr[:], AF.Sqrt)

            colr = persist.tile([128, NB], DT.float32)
            cold = persist.tile([128, NB], DT.float32)
            nc.scalar.activation(colr[:], colminT[:], AF.Relu)
            nc.scalar.activation(cold[:], colr[:], AF.Sqrt)

            junk = persist.tile([128, NB], DT.float32)
            outacc = persist.tile([128, 2], DT.float32)
            nc.vector.tensor_mul(junk[:], rowd[:], w1t[:])
            nc.vector.tensor_reduce(
                outacc[:, 0:1], junk[:], axis=mybir.AxisListType.X, op=ALU.add
            )
            junk2 = persist.tile([128, NB], DT.float32)
            nc.vector.tensor_mul(junk2[:], cold[:], w2t[:])
            nc.vector.tensor_reduce(
                outacc[:, 1:2], junk2[:], axis=mybir.AxisListType.X, op=ALU.add
            )
            nc.sync.dma_start(out_d[:], outacc[:])

    nc.compile()
    return nc


def _get_nc(repeat=1, parts="pe,act,dve"):
    key = ("nc", repeat, parts)
    if key not in _CACHE:
        _CACHE[key] = _build_program(repeat, parts)
    return _CACHE[key]


def _make_in_maps(set1, set2, w1, w2):
    in_maps = []
    for b in range(N_CORES):
        in_maps.append(
            {
                "xt": np.ascontiguousarray(set1[b].T, dtype=np.float32),
                "yt": np.ascontiguousarray(set2[b].T, dtype=np.float32),
                "w1t": np.ascontiguousarray(
                    w1[b].reshape(NB, 128).T, dtype=np.float32
                ),
                "w2t": np.ascontiguousarray(
                    w2[b].reshape(NB, 128).T, dtype=np.float32
                ),
            }
        )
    return in_maps


def kernel(set1, set2, w1, w2):
    global LAST_RESULTS
    set1 = np.asarray(set1, dtype=np.float32)
    set2 = np.asarray(set2, dtype=np.float32)
    w1 = np.asarray(w1, dtype=np.float32)
    w2 = np.asarray(w2, dtype=np.float32)

    nc = _get_nc()
    in_maps = _make_in_maps(set1, set2, w1, w2)
    res = run_bass_kernel_spmd(nc, in_maps, core_ids=list(range(N_CORES)))
    LAST_RESULTS = res

    total = 0.0
    for core_out in res.results:
        total += float(core_out["out"].astype(np.float64).sum())
    return np.float32(total / 2.0)


# revision 6
# speedup vs baseline: 1.3086x; 1.3086x over previous
"""Weighted Chamfer loss on Trainium2 (8 NeuronCores, batch-parallel).

Problem (per batch element b of 8):
    dist[i, j] = || set1[b, i] - set2[b, j] ||_2            (4096 x 4096, C=128)
    total = (sum_i w1[b,i] * min_j dist + sum_j w2[b,j] * min_i dist) / 2

Sharding: one batch element per NeuronCore (pure data parallel, no
collectives); the 8 per-core partial sums are added on the host.

Kernel strategy (vs the fp16 baseline):
  PE    : fp8e4 DoubleRow matmuls, K = 2 k-tiles x 66 rows = 128 channels
          + 4 spare rank-1 rows that bake -x2/2 (+ fp8 residual row) and
          -y2/2 (+ residual) into the same pass -> PSUM = -d^2/2 complete,
          at 0.5 cyc/row. One [128 x 4096] PSUM unit per x-block (all 8
          banks, 2-deep ping-pong), 8 matmuls of 512 cols each.
  ACT   : evacuates the unit with Identity(scale=-2) -> d2 fp16.
  DVE   : one fp16 tensor_reduce(min) over the unit -> row-min slot, and
          the column-min accumulator update (fp16 tensor_tensor min).
  Tail  : PE transposes of colacc + strided min reduce -> per-column mins;
          relu+sqrt; weighted sums; host adds the 8 per-core scalars.

The norms are computed from the *quantized* fp8 values (squares of fp8
are exact in fp16), so PSUM holds exactly -||x_q - y_q||^2/2 up to the
fp8 residual-row rounding (|err| <= ~0.25 on d2 values of ~170).
"""

import sys
from contextlib import ExitStack, nullcontext

import numpy as np

for _p in ("/opt/trn_rl_repo",):
    if _p not in sys.path:
        sys.path.insert(0, _p)

import concourse.bass as bass
import concourse.tile as tile
from concourse import bacc, masks, mybir
from concourse.bass_utils import run_bass_kernel_spmd

AF = mybir.ActivationFunctionType
ALU = mybir.AluOpType
DT = mybir.dt
PM = mybir.MatmulPerfMode

N_CORES = 8
N = 4096          # points per set per batch element
C = 128           # channels (contraction dim)
KP = C // 2 + 2   # 66: contraction rows per DoubleRow k-tile (64 ch + 2 bake)
NB = N // 128     # 32 row blocks of x
MMN = 512         # moving free dim per matmul (one fp32 PSUM bank)
NT = N // 128     # 32 transpose tiles

_CACHE = {}
LAST_RESULTS = None  # BassKernelResults of the most recent run (for profiling)


def _build_program(repeat=1, parts="pe,act,dve"):
    # tuning knob: "eNNNN" = ACT evac columns (rest go to DVE tensor_scalar)
    EA = N
    for p in parts.split(","):
        if p.startswith("e") and p[1:].isdigit():
            EA = int(p[1:])
    en_act = "act" in parts
    en_dve = "dve" in parts

    nc = bacc.Bacc(
        "TRN2", debug=False, target_bir_lowering=False, num_devices=N_CORES
    )
    xt_d = nc.dram_tensor("xt", [C, N], DT.float32, kind="ExternalInput").ap()
    yt_d = nc.dram_tensor("yt", [C, N], DT.float32, kind="ExternalInput").ap()
    w1t_d = nc.dram_tensor("w1t", [128, NB], DT.float32, kind="ExternalInput").ap()
    w2t_d = nc.dram_tensor("w2t", [128, NB], DT.float32, kind="ExternalInput").ap()
    out_d = nc.dram_tensor("out", [128, 2], DT.float32, kind="ExternalOutput").ap()

    with tile.TileContext(nc) as tc, ExitStack() as ctx:
        persist = ctx.enter_context(tc.tile_pool(name="persist", bufs=1))
        prep = ctx.enter_context(tc.tile_pool(name="prep", bufs=1))
        rows = ctx.enter_context(tc.tile_pool(name="rows", bufs=1))
        d2p = ctx.enter_context(tc.tile_pool(name="d2", bufs=4))
        psum = ctx.enter_context(tc.tile_pool(name="psum", bufs=2, space="PSUM"))

        # ---------------- persistent tiles ----------------
        # DoubleRow operands [KP, 2, N] stored as [KP, 2*N]:
        #   x tile0 rows: ch 0..63, then -x2/2 (fp8), res_x (fp8)
        #   x tile1 rows: ch 64..127, then 1, 1
        #   y tile0 rows: ch 0..63, then 1, 1
        #   y tile1 rows: ch 64..127, then -y2/2, res_y
        xq8 = persist.tile([KP, 2 * N], DT.float8e4)
        yq8 = persist.tile([KP, 2 * N], DT.float8e4)

        identity = persist.tile([128, 128], DT.float16)
        masks.make_identity(nc, identity[:])

        ones = persist.tile([C, 1], DT.float16)
        nc.gpsimd.memset(ones[:], 1.0)

        colacc = persist.tile([128, N], DT.float16)
        nc.gpsimd.memset(colacc[:], 60000.0)

        rm = persist.tile([128, NB], DT.float32)
        colminT = persist.tile([128, NB], DT.float32)

        w1t = persist.tile([128, NB], DT.float32)
        w2t = persist.tile([128, NB], DT.float32)
        nc.sync.dma_start(w1t[:], w1t_d[:])
        nc.sync.dma_start(w2t[:], w2t_d[:])

        # ---------------- prep: quantize + norms + assemble ----------------
        # spare rows default to 1.0; the norm rows overwrite their halves
        nc.vector.memset(xq8[C // 2 : KP, :], 1.0)
        nc.vector.memset(yq8[C // 2 : KP, :], 1.0)

        for src_d, q8big, half in ((xt_d, xq8, 0), (yt_d, yq8, 1)):
            stage = prep.tile([C, N], DT.float32, tag="stage")
            nc.sync.dma_start(stage[:], src_d[:])
            qfull = prep.tile([C, N], DT.float8e4, tag="qfull")
            nc.vector.tensor_copy(qfull[:], stage[:])
            h16 = prep.tile([C, N], DT.float16, tag="h16")
            nc.vector.tensor_copy(h16[:], qfull[:])
            sq = prep.tile([C, N], DT.float16, tag="sq")
            nc.scalar.activation(sq[:], h16[:], AF.Square)

            # -sum(sq)/2 over channels via ones-matmul into PSUM row 0
            mrow = rows.tile([1, N], DT.float32, tag="mrow")
            for hh in range(2):
                ps = psum.tile(
                    [128, N // 2], DT.float32, tag="unit", name=f"nps{half}{hh}"
                )
                for k in range(N // 2 // MMN):
                    c0 = k * MMN
                    nc.tensor.matmul(
                        ps[0:1, c0 : c0 + MMN],
                        ones[:],
                        sq[:, hh * (N // 2) + c0 : hh * (N // 2) + c0 + MMN],
                        start=True,
                        stop=True,
                    )
                nc.scalar.activation(
                    mrow[0:1, hh * (N // 2) : (hh + 1) * (N // 2)],
                    ps[0:1, :],
                    AF.Identity,
                    scale=-0.5,
                )

            m8 = rows.tile([1, N], DT.float8e4, tag="m8")
            nc.vector.tensor_copy(m8[:], mrow[:])
            mup = rows.tile([1, N], DT.float32, tag="mup")
            nc.vector.tensor_copy(mup[:], m8[:])
            resf = rows.tile([1, N], DT.float32, tag="resf")
            nc.vector.tensor_sub(resf[:], mrow[:], mup[:])
            r8 = rows.tile([1, N], DT.float8e4, tag="r8")
            nc.vector.tensor_copy(r8[:], resf[:])

            # assemble the [KP, 2, N] operand (SBUF->SBUF DMAs)
            nc.sync.dma_start(q8big[0 : C // 2, 0:N], qfull[0 : C // 2, :])
            nc.sync.dma_start(q8big[0 : C // 2, N : 2 * N], qfull[C // 2 : C, :])
            off = 0 if half == 0 else N
            nc.sync.dma_start(q8big[C // 2 : C // 2 + 1, off : off + N], m8[:])
            nc.sync.dma_start(q8big[C // 2 + 1 : KP, off : off + N], r8[:])

        xv = xq8[:].rearrange("p (two n) -> p two n", two=2)
        yv = yq8[:].rearrange("p (two n) -> p two n", two=2)

        if not en_dve:
            nc.gpsimd.memset(rm[:], 1.0)

        with tc.For_i(0, repeat, 1) if repeat > 1 else nullcontext():
            # ------- main loop: per x-block, two [128, 2048] PSUM units ------
            # evacuated into one [128, 4096] fp16 tile -> a single full-width
            # row-min reduce and a single full-width colacc min per block.
            for b in range(NB):
                d2 = d2p.tile([128, N], DT.float16, tag="d2")
                for h in range(2):
                    hc = h * (N // 2)
                    ps = psum.tile([128, N // 2], DT.float32, tag="unit")
                    for k in range(N // 2 // MMN):
                        c0 = k * MMN
                        nc.tensor.matmul(
                            ps[:, c0 : c0 + MMN],
                            xv[:, :, b * 128 : (b + 1) * 128],
                            yv[:, :, hc + c0 : hc + c0 + MMN],
                            start=True,
                            stop=True,
                            perf_mode=PM.DoubleRow,
                        )
                    if en_act:
                        ea = max(0, min(EA - hc, N // 2))
                        if ea > 0:
                            nc.scalar.activation(
                                d2[:, hc : hc + ea],
                                ps[:, 0:ea],
                                AF.Identity,
                                scale=-2.0,
                            )
                        if ea < N // 2:
                            nc.vector.tensor_scalar_mul(
                                d2[:, hc + ea : hc + N // 2],
                                ps[:, ea : N // 2],
                                -2.0,
                            )
                if en_dve:
                    # col accumulator first (reads full d2), then the row-min
                    # fold chain overwrites d2 in place; a final strided
                    # reduce collapses the last 512. (Plain full-width
                    # tensor_reduce runs at 1x on DVE -- folds are 2x.)
                    nc.vector.tensor_tensor(
                        colacc[:], d2[:], colacc[:], ALU.min
                    )
                    w = N // 2
                    while w >= 512:
                        nc.vector.tensor_tensor(
                            d2[:, 0:w], d2[:, 0:w], d2[:, w : 2 * w], ALU.min
                        )
                        w //= 2
                    w *= 2
                    nc.vector.tensor_reduce(
                        rm[:, b : b + 1],
                        d2[:, 0:w].rearrange("p (t c) -> p t c", c=min(w, 512)),
                        axis=mybir.AxisListType.XY,
                        op=ALU.min,
                    )

            # column mins: PE-transpose 128-blocks of colacc into PSUM
            # units, then one strided min-reduce per half.
            for h in range(2):
                hc = h * (N // 2)
                pst = psum.tile([128, N // 2], DT.float16, tag="unit", name="pst")
                for t in range(NT // 2):
                    nc.tensor.transpose(
                        pst[:, t * 128 : (t + 1) * 128],
                        colacc[:, hc + t * 128 : hc + (t + 1) * 128],
                        identity[:],
                    )
                nc.vector.tensor_reduce(
                    colminT[:, h * (NB // 2) : (h + 1) * (NB // 2)],
                    pst[:].rearrange("p (t c) -> p t c", c=128),
                    axis=mybir.AxisListType.X,
                    op=ALU.min,
                )

            # ---------------- tail ----------------
            rowr = persist.tile([128, NB], DT.float32)
            rowd = persist.tile([128, NB], DT.float32)
            nc.scalar.activation(rowr[:], rm[:], AF.Relu)
            nc.scalar.activation(rowd[:], rowr[:], AF.Sqrt)

            colr = persist.tile([128, NB], DT.float32)
            cold = persist.tile([128, NB], DT.float32)
            nc.scalar.activation(colr[:], colminT[:], AF.Relu)
            nc.scalar.activation(cold[:], colr[:], AF.Sqrt)

            junk = persist.tile([128, NB], DT.float32)
            outacc = persist.tile([128, 2], DT.float32)
            nc.vector.tensor_mul(junk[:], rowd[:], w1t[:])
            nc.vector.tensor_reduce(
                outacc[:, 0:1], junk[:], axis=mybir.AxisListType.X, op=ALU.add
            )
            junk2 = persist.tile([128, NB], DT.float32)
            nc.vector.tensor_mul(junk2[:], cold[:], w2t[:])
            nc.vector.tensor_reduce(
                outacc[:, 1:2], junk2[:], axis=mybir.AxisListType.X, op=ALU.add
            )
            nc.sync.dma_start(out_d[:], outacc[:])

    nc.compile()
    return nc


def _get_nc(repeat=1, parts="pe,act,dve"):
    key = ("nc", repeat, parts)
    if key not in _CACHE:
        _CACHE[key] = _build_program(repeat, parts)
    return _CACHE[key]


def _make_in_maps(set1, set2, w1, w2):
    in_maps = []
    for b in range(N_CORES):
        in_maps.append(
            {
                "xt": np.ascontiguousarray(set1[b].T, dtype=np.float32),
                "yt": np.ascontiguousarray(set2[b].T, dtype=np.float32),
                "w1t": np.ascontiguousarray(
                    w1[b].reshape(NB, 128).T, dtype=np.float32
                ),
                "w2t": np.ascontiguousarray(
                    w2[b].reshape(NB, 128).T, dtype=np.float32
                ),
            }
        )
    return in_maps


def kernel(set1, set2, w1, w2):
    global LAST_RESULTS
    set1 = np.asarray(set1, dtype=np.float32)
    set2 = np.asarray(set2, dtype=np.float32)
    w1 = np.asarray(w1, dtype=np.float32)
    w2 = np.asarray(w2, dtype=np.float32)

    nc = _get_nc()
    in_maps = _make_in_maps(set1, set2, w1, w2)
    res = run_bass_kernel_spmd(nc, in_maps, core_ids=list(range(N_CORES)))
    LAST_RESULTS = res

    total = 0.0
    for core_out in res.results:
        total += float(core_out["out"].astype(np.float64).sum())
    return np.float32(total / 2.0)
